# revision 6
# baseline (speedup 1.0000x reference)
"""Trainium2 Bass kernel for nn_LocalPoolPointnet (LocalPoolPointnet, 8 cores).

Strategy (chosen sharding): each of the 4 samples is split across 2 cores at a
scatter-bin boundary, after sorting that sample's points by flat bin index
(host-side prep, part of sharding). Each core therefore owns a set of COMPLETE
bins, so no cross-core communication is needed: local-max pooling and the final
scatter-mean are computed with segmented scans along the sorted point stream.

Device pipeline (feature-major: 128 features on partitions, points on free):
  - MLP resblocks as PE matmuls (bf16, fp32 PSUM accumulate)
  - pool_local_max = fwd+bwd masked max scans (tensor_tensor_scan,
    op0=min with +/-BIG segment mask, op1=max), pooled = max(fwd, bwd)
  - final scatter_mean = masked sum scan; at each bin's last point the running
    sum is the bin total; multiply by host-provided 1/count, transpose the
    chunk to point-major, and indirect-DMA the segment-end rows to the
    per-bin output table. Non-end rows go to a dummy row.
"""

import os
import numpy as np
from contextlib import ExitStack

R = 128
R2 = R * R
H = 128
NB = 5
BIG = 1e30

STRIP = 512
NSTRIP_SEG = int(os.environ.get("KERNEL_NSTRIP", 26))  # strips per segment
SEG_M = STRIP * NSTRIP_SEG    # 13312 points per segment
NSEG = int(os.environ.get("KERNEL_NSEG", 4))           # segments per core
N_CAP = SEG_M * NSEG          # per-core padded point capacity
NCH = SEG_M // 128            # 104 point-chunks per segment
NCORES = 8


# --------------------------------------------------------------------------
# host-side prep: sort by bin, split across cores/segments, build metadata
# --------------------------------------------------------------------------

def _host_prep(inputs):
    b, n, _ = inputs.shape
    cores = []
    for s in range(b):
        pts = np.asarray(inputs[s], np.float32)
        ij = np.clip((pts[:, :2] * R).astype(np.int32), 0, R - 1)
        idx = ij[:, 0] + R * ij[:, 1]
        order = np.argsort(idx, kind="stable")
        idx_s = idx[order]
        pts_s = pts[order]
        half = n // 2
        split_pos = int(np.searchsorted(idx_s, idx_s[half]))
        for (lo, hi) in [(0, split_pos), (split_pos, n)]:
            ci, cp = idx_s[lo:hi], pts_s[lo:hi]
            npts = hi - lo
            assert npts <= N_CAP, f"core overflow {npts} > {N_CAP}"
            seg_bounds = [0]
            for k in range(1, NSEG):
                tgt = min(npts - 1, round(npts * k / NSEG))
                seg_bounds.append(int(np.searchsorted(ci, ci[tgt])))
            seg_bounds.append(npts)

            pos = np.zeros((4, N_CAP), np.float32)
            maxmask = np.full((N_CAP + 1,), -BIG, np.float32)
            summask = np.zeros((N_CAP,), np.float32)
            wrow = np.zeros((N_CAP,), np.float32)
            targets = np.full((N_CAP,), R2, np.int32)
            for k in range(NSEG):
                a, e = seg_bounds[k], seg_bounds[k + 1]
                m = e - a
                assert m <= SEG_M, f"segment overflow {m} > {SEG_M}"
                o = k * SEG_M
                seg_idx = ci[a:e]
                pos[:3, o:o + m] = cp[a:e].T
                same = np.zeros(m, bool)
                same[1:] = seg_idx[1:] == seg_idx[:-1]
                maxmask[o:o + m] = np.where(same, BIG, -BIG)
                summask[o:o + m] = same.astype(np.float32)
                is_end = np.ones(m, bool)
                is_end[:-1] = seg_idx[1:] != seg_idx[:-1]
                _, inv, cnt = np.unique(seg_idx, return_inverse=True,
                                        return_counts=True)
                wrow[o:o + m] = np.where(is_end, 1.0 / cnt[inv], 0.0)
                targets[o:o + m] = np.where(is_end, seg_idx, R2)
            owned = np.unique(ci)
            cores.append(dict(sample=s, pos=pos, maxmask=maxmask,
                              summask=summask, wrow=wrow, targets=targets,
                              owned=owned))
    return cores


def _pack_weights(W):
    """Pack reference weights into the device layouts (bf16/fp32)."""
    bf = np.dtype("bfloat16") if hasattr(np, "bfloat16") else None
    import ml_dtypes
    bf16 = ml_dtypes.bfloat16

    w0 = np.zeros((128, NB * 2 * 128), np.float32)
    ws = np.zeros((128, NB * 2 * 128), np.float32)
    w1 = np.zeros((128, NB * 128), np.float32)
    for blk in range(NB):
        w0[:, (2 * blk) * 128:(2 * blk + 1) * 128] = W["W0"][blk][:128, :]
        w0[:, (2 * blk + 1) * 128:(2 * blk + 2) * 128] = W["W0"][blk][128:, :]
        ws[:, (2 * blk) * 128:(2 * blk + 1) * 128] = W["Ws"][blk][:128, :]
        ws[:, (2 * blk + 1) * 128:(2 * blk + 2) * 128] = W["Ws"][blk][128:, :]
        w1[:, blk * 128:(blk + 1) * 128] = W["W1"][blk]

    wpos = np.zeros((4, 256), np.float32)
    wpos[:3] = W["Wpos"]
    # block0 shortcut folded through the (linear) position encoder:
    # x0 @ Ws0 = pos @ (Wpos @ Ws0) + bpos @ Ws0
    wps = np.zeros((4, 128), np.float32)
    wps[:3] = W["Wpos"] @ W["Ws"][0]
    bps = W["bpos"] @ W["Ws"][0]

    biases = np.zeros((128, 2 + 2 * NB + 1), np.float32)
    biases[:, 0] = W["bpos"][:128]
    biases[:, 1] = W["bpos"][128:]
    for blk in range(NB):
        biases[:, 2 + blk] = W["b0"][blk]
        biases[:, 2 + NB + blk] = W["b1"][blk]
    biases[:, 2 + NB] += bps          # fold folded-shortcut bias into b1[0]
    biases[:, 2 + 2 * NB] = W["bc"]

    return dict(
        w0=w0.astype(bf16), ws=ws.astype(bf16), w1=w1.astype(bf16),
        wpos=wpos.astype(bf16), wps=wps.astype(bf16),
        wc=np.ascontiguousarray(W["Wc"]).astype(bf16),
        biases=biases,
    )


def _make_core_inputs(core, packed):
    import ml_dtypes
    bf16 = ml_dtypes.bfloat16
    mm = np.broadcast_to(core["maxmask"][None, :], (128, N_CAP + 1))
    sm = np.broadcast_to(core["summask"][None, :], (128, N_CAP))
    return dict(
        pos=core["pos"].astype(bf16),
        maxmask=np.ascontiguousarray(mm).astype(bf16),
        summask=np.ascontiguousarray(sm).astype(bf16),
        wcol=np.ascontiguousarray(
            core["wrow"].reshape(-1, 128).T).astype(np.float32),
        targets=np.ascontiguousarray(
            core["targets"].reshape(-1, 128).T).astype(np.int32),
        **packed,
    )


# --------------------------------------------------------------------------
# device kernel
# --------------------------------------------------------------------------

def _build_kernel(nseg=NSEG):
    import concourse.bass as bass
    import concourse.tile as tile
    import concourse.mybir as mybir
    from concourse import bacc

    BF16 = mybir.dt.bfloat16
    FP32 = mybir.dt.float32
    I32 = mybir.dt.int32
    AF = mybir.ActivationFunctionType
    ALU = mybir.AluOpType
    n_cap = SEG_M * nseg
    nch_total = (n_cap) // 128

    nc = bacc.Bacc("TRN2", target_bir_lowering=False, debug=False,
                   num_devices=NCORES)

    pos_d = nc.dram_tensor("pos", [4, n_cap], BF16, kind="ExternalInput")
    mm_d = nc.dram_tensor("maxmask", [128, n_cap + 1], BF16,
                          kind="ExternalInput")
    sm_d = nc.dram_tensor("summask", [128, n_cap], BF16, kind="ExternalInput")
    wcol_d = nc.dram_tensor("wcol", [128, nch_total], FP32,
                            kind="ExternalInput")
    tg_d = nc.dram_tensor("targets", [128, nch_total], I32,
                          kind="ExternalInput")
    w0_d = nc.dram_tensor("w0", [128, NB * 2 * 128], BF16,
                          kind="ExternalInput")
    ws_d = nc.dram_tensor("ws", [128, NB * 2 * 128], BF16,
                          kind="ExternalInput")
    w1_d = nc.dram_tensor("w1", [128, NB * 128], BF16, kind="ExternalInput")
    wpos_d = nc.dram_tensor("wpos", [4, 256], BF16, kind="ExternalInput")
    wps_d = nc.dram_tensor("wps", [4, 128], BF16, kind="ExternalInput")
    wc_d = nc.dram_tensor("wc", [128, 128], BF16, kind="ExternalInput")
    bias_d = nc.dram_tensor("biases", [128, 2 + 2 * NB + 1], FP32,
                            kind="ExternalInput")
    table_d = nc.dram_tensor("table", [R2 + 1, 128], FP32,
                             kind="ExternalOutput")

    with tile.TileContext(nc) as tc, ExitStack() as ctx:
        const = ctx.enter_context(tc.tile_pool(name="const", bufs=1))
        segio = ctx.enter_context(tc.tile_pool(name="segio", bufs=1))
        nets = ctx.enter_context(tc.tile_pool(name="nets", bufs=1))
        scanp = ctx.enter_context(tc.tile_pool(name="scanp", bufs=1))
        work = ctx.enter_context(tc.tile_pool(name="work", bufs=3))
        csump = ctx.enter_context(tc.tile_pool(name="csum", bufs=3))
        psum = ctx.enter_context(tc.tile_pool(name="psum", bufs=3,
                                              space="PSUM"))
        psum_tr = ctx.enter_context(tc.tile_pool(name="psum_tr", bufs=2,
                                                 space="PSUM"))

        # ---- constants ----
        w0_t = const.tile([128, NB * 2 * 128], BF16)
        ws_t = const.tile([128, NB * 2 * 128], BF16)
        w1_t = const.tile([128, NB * 128], BF16)
        wpos_t = const.tile([4, 256], BF16)
        wps_t = const.tile([4, 128], BF16)
        wc_t = const.tile([128, 128], BF16)
        bias_t = const.tile([128, 2 + 2 * NB + 1], FP32)
        ident = const.tile([128, 128], FP32)
        zeros_t = const.tile([128, 512], FP32)
        nc.sync.dma_start(out=w0_t[:], in_=w0_d.ap())
        nc.sync.dma_start(out=ws_t[:], in_=ws_d.ap())
        nc.sync.dma_start(out=w1_t[:], in_=w1_d.ap())
        nc.sync.dma_start(out=wpos_t[:], in_=wpos_d.ap())
        nc.sync.dma_start(out=wps_t[:], in_=wps_d.ap())
        nc.sync.dma_start(out=wc_t[:], in_=wc_d.ap())
        nc.sync.dma_start(out=bias_t[:], in_=bias_d.ap())
        from concourse.masks import make_identity
        make_identity(nc, ident[:])
        nc.vector.memset(zeros_t[:], 0.0)

        def bias_ap(col):
            return bias_t[:, col:col + 1]

        # zero-init the output table: view [16384+1, 128] rows as
        # [128 part, 16 a, 8 chunks...] -> 8 chunked DMAs + last row
        tbl_main = table_d.ap()[0:R2, :].rearrange("(a p) f -> p a f", p=128)
        for i in range(32):
            nc.sync.dma_start(out=tbl_main[:, i * 4:(i + 1) * 4, :],
                              in_=zeros_t[:].rearrange("p (a f) -> p a f",
                                                       f=128))
        nc.sync.dma_start(out=table_d.ap()[R2:R2 + 1, :],
                          in_=zeros_t[0:1, 0:128])

        for seg in range(nseg):
            o = seg * SEG_M
            mm_s = segio.tile([128, SEG_M + 1], BF16, tag="maxmask")
            sm_s = segio.tile([128, SEG_M], BF16, tag="summask")
            wcol_s = segio.tile([128, NCH], FP32, tag="wcol")
            tg_s = segio.tile([128, NCH], I32, tag="targets")
            nc.sync.dma_start(out=mm_s[:], in_=mm_d.ap()[:, o:o + SEG_M + 1])
            nc.sync.dma_start(out=sm_s[:], in_=sm_d.ap()[:, o:o + SEG_M])
            nc.sync.dma_start(out=wcol_s[:],
                              in_=wcol_d.ap()[:, seg * NCH:(seg + 1) * NCH])
            nc.sync.dma_start(out=tg_s[:],
                              in_=tg_d.ap()[:, seg * NCH:(seg + 1) * NCH])

            netA = nets.tile([128, SEG_M], BF16, tag="netA")
            netB = nets.tile([128, SEG_M], BF16, tag="netB")

            # ---- block 0 (no pooling; shortcut folded through pos) ----
            for s in range(NSTRIP_SEG):
                sl = slice(s * STRIP, (s + 1) * STRIP)
                pos_s = work.tile([4, STRIP], BF16, tag="pos")
                nc.sync.dma_start(out=pos_s[:],
                                  in_=pos_d.ap()[:, o + s * STRIP:
                                                 o + (s + 1) * STRIP])
                ps_lo = psum.tile([128, STRIP], FP32, tag="ps_a")
                ps_hi = psum.tile([128, STRIP], FP32, tag="ps_b")
                nc.tensor.matmul(out=ps_lo[:], lhsT=wpos_t[:, 0:128],
                                 rhs=pos_s[:], start=True, stop=True)
                nc.tensor.matmul(out=ps_hi[:], lhsT=wpos_t[:, 128:256],
                                 rhs=pos_s[:], start=True, stop=True)
                rx_lo = work.tile([128, STRIP], BF16, tag="rx_lo")
                rx_hi = work.tile([128, STRIP], BF16, tag="rx_hi")
                nc.scalar.activation(out=rx_lo[:], in_=ps_lo[:], func=AF.Relu,
                                     bias=bias_ap(0))
                nc.scalar.activation(out=rx_hi[:], in_=ps_hi[:], func=AF.Relu,
                                     bias=bias_ap(1))
                ps_n = psum.tile([128, STRIP], FP32, tag="ps_a")
                nc.tensor.matmul(out=ps_n[:], lhsT=w0_t[:, 0:128],
                                 rhs=rx_lo[:], start=True, stop=False)
                nc.tensor.matmul(out=ps_n[:], lhsT=w0_t[:, 128:256],
                                 rhs=rx_hi[:], start=False, stop=True)
                rn = work.tile([128, STRIP], BF16, tag="rn")
                nc.scalar.activation(out=rn[:], in_=ps_n[:], func=AF.Relu,
                                     bias=bias_ap(2))
                ps_o = psum.tile([128, STRIP], FP32, tag="ps_b")
                nc.tensor.matmul(out=ps_o[:], lhsT=w1_t[:, 0:128],
                                 rhs=rn[:], start=True, stop=False)
                nc.tensor.matmul(out=ps_o[:], lhsT=wps_t[:],
                                 rhs=pos_s[:], start=False, stop=True)
                nc.scalar.activation(out=netA[:, sl], in_=ps_o[:],
                                     func=AF.Identity, bias=bias_ap(2 + NB))

            # ---- blocks 1..4 ----
            cur, nxt = netA, netB
            for blk in range(1, NB):
                fwd = scanp.tile([128, SEG_M], BF16, tag="fwd")
                bwd = scanp.tile([128, SEG_M], BF16, tag="bwd")
                nc.vector.tensor_tensor_scan(
                    out=fwd[:], data0=mm_s[:, 0:SEG_M], data1=cur[:],
                    initial=-BIG, op0=ALU.min, op1=ALU.max)
                nc.vector.tensor_tensor_scan(
                    out=bwd[:, ::-1], data0=mm_s[:, 1:SEG_M + 1][:, ::-1],
                    data1=cur[:, ::-1],
                    initial=-BIG, op0=ALU.min, op1=ALU.max)
                c0 = 2 * blk * 128
                c1 = (2 * blk + 1) * 128
                for s in range(NSTRIP_SEG):
                    sl = slice(s * STRIP, (s + 1) * STRIP)
                    pooled = work.tile([128, STRIP], BF16, tag="pooled")
                    rp = work.tile([128, STRIP], BF16, tag="rp")
                    rn_prev = work.tile([128, STRIP], BF16, tag="rn_prev")
                    nc.vector.tensor_tensor(out=pooled[:], in0=fwd[:, sl],
                                            in1=bwd[:, sl], op=ALU.max)
                    nc.gpsimd.tensor_scalar_max(rp[:], pooled[:], 0.0)
                    nc.gpsimd.tensor_scalar_max(rn_prev[:], cur[:, sl], 0.0)
                    ps_n = psum.tile([128, STRIP], FP32, tag="ps_a")
                    nc.tensor.matmul(out=ps_n[:], lhsT=w0_t[:, c0:c0 + 128],
                                     rhs=rn_prev[:], start=True, stop=False)
                    nc.tensor.matmul(out=ps_n[:], lhsT=w0_t[:, c1:c1 + 128],
                                     rhs=rp[:], start=False, stop=True)
                    rn = work.tile([128, STRIP], BF16, tag="rn")
                    nc.scalar.activation(out=rn[:], in_=ps_n[:], func=AF.Relu,
                                         bias=bias_ap(2 + blk))
                    ps_o = psum.tile([128, STRIP], FP32, tag="ps_b")
                    nc.tensor.matmul(out=ps_o[:],
                                     lhsT=w1_t[:, blk * 128:(blk + 1) * 128],
                                     rhs=rn[:], start=True, stop=False)
                    nc.tensor.matmul(out=ps_o[:], lhsT=ws_t[:, c0:c0 + 128],
                                     rhs=cur[:, sl], start=False, stop=False)
                    nc.tensor.matmul(out=ps_o[:], lhsT=ws_t[:, c1:c1 + 128],
                                     rhs=pooled[:], start=False, stop=True)
                    nc.scalar.activation(out=nxt[:, sl], in_=ps_o[:],
                                         func=AF.Identity,
                                         bias=bias_ap(2 + NB + blk))
                cur, nxt = nxt, cur

            # ---- final projection + segmented mean + scatter ----
            prev_csum = None
            for s in range(NSTRIP_SEG):
                sl = slice(s * STRIP, (s + 1) * STRIP)
                rn = work.tile([128, STRIP], BF16, tag="rn_f")
                nc.gpsimd.tensor_scalar_max(rn[:], cur[:, sl], 0.0)
                ps_c = psum.tile([128, STRIP], FP32, tag="ps_a")
                nc.tensor.matmul(out=ps_c[:], lhsT=wc_t[:], rhs=rn[:],
                                 start=True, stop=True)
                c_t = work.tile([128, STRIP], BF16, tag="c_t")
                nc.scalar.activation(out=c_t[:], in_=ps_c[:],
                                     func=AF.Identity,
                                     bias=bias_ap(2 + 2 * NB))
                csum = csump.tile([128, STRIP], FP32, tag="csum")
                nc.vector.tensor_tensor_scan(
                    out=csum[:], data0=sm_s[:, sl], data1=c_t[:],
                    initial=(0.0 if prev_csum is None
                             else prev_csum[:, STRIP - 1:STRIP]),
                    op0=ALU.mult, op1=ALU.add)
                prev_csum = csum
                for ch in range(4):
                    chg = s * 4 + ch
                    pt = psum_tr.tile([128, 128], FP32, tag="ptr")
                    nc.tensor.transpose(
                        out=pt[:], in_=csum[:, ch * 128:(ch + 1) * 128],
                        identity=ident[:])
                    srow = work.tile([128, 128], FP32, tag="srow")
                    nc.vector.tensor_scalar_mul(srow[:], pt[:],
                                                wcol_s[:, chg:chg + 1])
                    nc.gpsimd.indirect_dma_start(
                        out=table_d.ap(),
                        out_offset=bass.IndirectOffsetOnAxis(
                            ap=tg_s[:, chg:chg + 1], axis=0),
                        in_=srow[:], in_offset=None)

    nc.compile()
    return nc


# --------------------------------------------------------------------------
# public entry point
# --------------------------------------------------------------------------

_CACHED = {}


def kernel(inputs, Wpos, bpos, W0, b0, W1, b1, Ws, Wc, bc):
    W = dict(Wpos=np.asarray(Wpos, np.float32), bpos=np.asarray(bpos, np.float32),
             W0=np.asarray(W0, np.float32), b0=np.asarray(b0, np.float32),
             W1=np.asarray(W1, np.float32), b1=np.asarray(b1, np.float32),
             Ws=np.asarray(Ws, np.float32), Wc=np.asarray(Wc, np.float32),
             bc=np.asarray(bc, np.float32))
    inputs = np.asarray(inputs, np.float32)
    b = inputs.shape[0]

    cores = _host_prep(inputs)
    packed = _pack_weights(W)
    in_maps = [_make_core_inputs(c, packed) for c in cores]

    # bass_utils' axon trace path imports antenv.axon_hooks, which may be
    # missing on this image; provide a minimal stand-in so trace requests
    # degrade gracefully instead of raising.
    try:
        from antenv import axon_hooks  # noqa: F401
    except ImportError:
        import sys
        import types
        import antenv
        mod = types.ModuleType("antenv.axon_hooks")
        mod._hook = None
        mod.set_axon_ntff_profile_hook = lambda h: setattr(mod, "_hook", h)
        mod.get_axon_ntff_profile_hook = lambda: mod._hook
        sys.modules["antenv.axon_hooks"] = mod
        antenv.axon_hooks = mod

    from concourse.bass_utils import run_bass_kernel_spmd
    if "nc" not in _CACHED:
        _CACHED["nc"] = _build_kernel()
    nc = _CACHED["nc"]

    trace = bool(int(os.environ.get("KERNEL_TRACE", "0")))
    res = run_bass_kernel_spmd(nc, in_maps, core_ids=list(range(NCORES)),
                               trace=trace)
    if trace and res.exec_time_ns is not None:
        print(f"HW exec time: {res.exec_time_ns} ns")
        _CACHED["exec_time_ns"] = res.exec_time_ns
        _CACHED["trace"] = res.instructions_and_trace

    out = np.zeros((b, H, R2), np.float32)
    for core, r in zip(cores, res.results):
        table = np.asarray(r["table"])
        owned = core["owned"]
        out[core["sample"]][:, owned] = table[owned].T
    return out.reshape(b, H, R, R)


# revision 7
# speedup vs baseline: 2.4234x; 2.4234x over previous
"""Trainium2 Bass kernel for nn_LocalPoolPointnet (LocalPoolPointnet, 8 cores).

Strategy (chosen sharding): each of the 4 samples is split across 2 cores at a
scatter-bin boundary, after sorting that sample's points by flat bin index
(host-side prep, part of sharding). Each core therefore owns a set of COMPLETE
bins, so no cross-core communication is needed: local-max pooling and the final
scatter-mean are computed with segmented scans along the sorted point stream.

Device pipeline (feature-major: 128 features on partitions, points on free):
  - MLP resblocks as PE matmuls (bf16, fp32 PSUM accumulate)
  - pool_local_max = fwd+bwd masked max scans (tensor_tensor_scan,
    op0=min with +/-BIG segment mask, op1=max), pooled = max(fwd, bwd)
  - final scatter_mean = masked sum scan; at each bin's last point the running
    sum is the bin total; multiply by host-provided 1/count, transpose the
    chunk to point-major, and indirect-DMA the segment-end rows to the
    per-bin output table. Non-end rows go to a dummy row.
"""

import os
import numpy as np
from contextlib import ExitStack

R = 128
R2 = R * R
H = 128
NB = 5
BIG = 1e30

STRIP = 512
NSTRIP_SEG = int(os.environ.get("KERNEL_NSTRIP", 26))  # strips per segment
SEG_M = STRIP * NSTRIP_SEG    # 13312 points per segment
NSEG = int(os.environ.get("KERNEL_NSEG", 4))           # segments per core
N_CAP = SEG_M * NSEG          # per-core padded point capacity
NCH = SEG_M // 128            # 104 point-chunks per segment
NCORES = 8


# --------------------------------------------------------------------------
# host-side prep: sort by bin, split across cores/segments, build metadata
# --------------------------------------------------------------------------

def _host_prep(inputs):
    b, n, _ = inputs.shape
    cores = []
    for s in range(b):
        pts = np.asarray(inputs[s], np.float32)
        ij = np.clip((pts[:, :2] * R).astype(np.int32), 0, R - 1)
        idx = ij[:, 0] + R * ij[:, 1]
        order = np.argsort(idx, kind="stable")
        idx_s = idx[order]
        pts_s = pts[order]
        half = n // 2
        split_pos = int(np.searchsorted(idx_s, idx_s[half]))
        for (lo, hi) in [(0, split_pos), (split_pos, n)]:
            ci, cp = idx_s[lo:hi], pts_s[lo:hi]
            npts = hi - lo
            assert npts <= N_CAP, f"core overflow {npts} > {N_CAP}"
            seg_bounds = [0]
            for k in range(1, NSEG):
                tgt = min(npts - 1, round(npts * k / NSEG))
                seg_bounds.append(int(np.searchsorted(ci, ci[tgt])))
            seg_bounds.append(npts)

            pos = np.zeros((4, N_CAP), np.float32)
            maxmask = np.full((N_CAP + 1,), -BIG, np.float32)
            summask = np.zeros((N_CAP,), np.float32)
            wrow = np.zeros((N_CAP,), np.float32)
            targets = np.full((N_CAP,), R2, np.int32)
            for k in range(NSEG):
                a, e = seg_bounds[k], seg_bounds[k + 1]
                m = e - a
                assert m <= SEG_M, f"segment overflow {m} > {SEG_M}"
                o = k * SEG_M
                seg_idx = ci[a:e]
                pos[:3, o:o + m] = cp[a:e].T
                same = np.zeros(m, bool)
                same[1:] = seg_idx[1:] == seg_idx[:-1]
                maxmask[o:o + m] = np.where(same, BIG, -BIG)
                summask[o:o + m] = same.astype(np.float32)
                is_end = np.ones(m, bool)
                is_end[:-1] = seg_idx[1:] != seg_idx[:-1]
                _, inv, cnt = np.unique(seg_idx, return_inverse=True,
                                        return_counts=True)
                wrow[o:o + m] = np.where(is_end, 1.0 / cnt[inv], 0.0)
                targets[o:o + m] = np.where(is_end, seg_idx, R2)
            owned = np.unique(ci)
            cores.append(dict(sample=s, pos=pos, maxmask=maxmask,
                              summask=summask, wrow=wrow, targets=targets,
                              owned=owned))
    return cores


def _pack_weights(W):
    """Pack reference weights into the device layouts (bf16/fp32)."""
    bf = np.dtype("bfloat16") if hasattr(np, "bfloat16") else None
    import ml_dtypes
    bf16 = ml_dtypes.bfloat16

    w0 = np.zeros((128, NB * 2 * 128), np.float32)
    ws = np.zeros((128, NB * 2 * 128), np.float32)
    w1 = np.zeros((128, NB * 128), np.float32)
    for blk in range(NB):
        w0[:, (2 * blk) * 128:(2 * blk + 1) * 128] = W["W0"][blk][:128, :]
        w0[:, (2 * blk + 1) * 128:(2 * blk + 2) * 128] = W["W0"][blk][128:, :]
        ws[:, (2 * blk) * 128:(2 * blk + 1) * 128] = W["Ws"][blk][:128, :]
        ws[:, (2 * blk + 1) * 128:(2 * blk + 2) * 128] = W["Ws"][blk][128:, :]
        w1[:, blk * 128:(blk + 1) * 128] = W["W1"][blk]

    wpos = np.zeros((4, 256), np.float32)
    wpos[:3] = W["Wpos"]
    # block0 shortcut folded through the (linear) position encoder:
    # x0 @ Ws0 = pos @ (Wpos @ Ws0) + bpos @ Ws0
    wps = np.zeros((4, 128), np.float32)
    wps[:3] = W["Wpos"] @ W["Ws"][0]
    bps = W["bpos"] @ W["Ws"][0]

    biases = np.zeros((128, 2 + 2 * NB + 1), np.float32)
    biases[:, 0] = W["bpos"][:128]
    biases[:, 1] = W["bpos"][128:]
    for blk in range(NB):
        biases[:, 2 + blk] = W["b0"][blk]
        biases[:, 2 + NB + blk] = W["b1"][blk]
    biases[:, 2 + NB] += bps          # fold folded-shortcut bias into b1[0]
    biases[:, 2 + 2 * NB] = W["bc"]

    return dict(
        w0=w0.astype(bf16), ws=ws.astype(bf16), w1=w1.astype(bf16),
        wpos=wpos.astype(bf16), wps=wps.astype(bf16),
        wc=np.ascontiguousarray(W["Wc"]).astype(bf16),
        biases=biases,
    )


def _make_core_inputs(core, packed):
    import ml_dtypes
    bf16 = ml_dtypes.bfloat16
    mm = np.broadcast_to(core["maxmask"][None, :], (128, N_CAP + 1))
    sm = np.broadcast_to(core["summask"][None, :], (128, N_CAP))
    return dict(
        pos=core["pos"].astype(bf16),
        maxmask=np.ascontiguousarray(mm).astype(bf16),
        summask=np.ascontiguousarray(sm).astype(bf16),
        wcol=np.ascontiguousarray(
            core["wrow"].reshape(-1, 128).T).astype(np.float32),
        targets=np.ascontiguousarray(
            core["targets"].reshape(-1, 128).T).astype(np.int32),
        **packed,
    )


# --------------------------------------------------------------------------
# device kernel
# --------------------------------------------------------------------------

def _build_kernel(nseg=NSEG):
    import concourse.bass as bass
    import concourse.tile as tile
    import concourse.mybir as mybir
    from concourse import bacc

    BF16 = mybir.dt.bfloat16
    FP32 = mybir.dt.float32
    I32 = mybir.dt.int32
    AF = mybir.ActivationFunctionType
    ALU = mybir.AluOpType
    n_cap = SEG_M * nseg
    nch_total = (n_cap) // 128

    nc = bacc.Bacc("TRN2", target_bir_lowering=False, debug=False,
                   num_devices=NCORES)

    pos_d = nc.dram_tensor("pos", [4, n_cap], BF16, kind="ExternalInput")
    mm_d = nc.dram_tensor("maxmask", [128, n_cap + 1], BF16,
                          kind="ExternalInput")
    sm_d = nc.dram_tensor("summask", [128, n_cap], BF16, kind="ExternalInput")
    wcol_d = nc.dram_tensor("wcol", [128, nch_total], FP32,
                            kind="ExternalInput")
    tg_d = nc.dram_tensor("targets", [128, nch_total], I32,
                          kind="ExternalInput")
    w0_d = nc.dram_tensor("w0", [128, NB * 2 * 128], BF16,
                          kind="ExternalInput")
    ws_d = nc.dram_tensor("ws", [128, NB * 2 * 128], BF16,
                          kind="ExternalInput")
    w1_d = nc.dram_tensor("w1", [128, NB * 128], BF16, kind="ExternalInput")
    wpos_d = nc.dram_tensor("wpos", [4, 256], BF16, kind="ExternalInput")
    wps_d = nc.dram_tensor("wps", [4, 128], BF16, kind="ExternalInput")
    wc_d = nc.dram_tensor("wc", [128, 128], BF16, kind="ExternalInput")
    bias_d = nc.dram_tensor("biases", [128, 2 + 2 * NB + 1], FP32,
                            kind="ExternalInput")
    table_d = nc.dram_tensor("table", [R2 + 1, 128], FP32,
                             kind="ExternalOutput")

    with tile.TileContext(nc) as tc, ExitStack() as ctx:
        const = ctx.enter_context(tc.tile_pool(name="const", bufs=1))
        segio = ctx.enter_context(tc.tile_pool(name="segio", bufs=1))
        nets = ctx.enter_context(tc.tile_pool(name="nets", bufs=1))
        scanp = ctx.enter_context(tc.tile_pool(name="scanp", bufs=1))
        work = ctx.enter_context(tc.tile_pool(name="work", bufs=3))
        csump = ctx.enter_context(tc.tile_pool(name="csum", bufs=3))
        psum = ctx.enter_context(tc.tile_pool(name="psum", bufs=3,
                                              space="PSUM"))
        psum_tr = ctx.enter_context(tc.tile_pool(name="psum_tr", bufs=2,
                                                 space="PSUM"))

        # ---- constants ----
        w0_t = const.tile([128, NB * 2 * 128], BF16)
        ws_t = const.tile([128, NB * 2 * 128], BF16)
        w1_t = const.tile([128, NB * 128], BF16)
        wpos_t = const.tile([4, 256], BF16)
        wps_t = const.tile([4, 128], BF16)
        wc_t = const.tile([128, 128], BF16)
        bias_t = const.tile([128, 2 + 2 * NB + 1], FP32)
        ident = const.tile([128, 128], FP32)
        zeros_t = const.tile([128, 512], FP32)
        nc.sync.dma_start(out=w0_t[:], in_=w0_d.ap())
        nc.sync.dma_start(out=ws_t[:], in_=ws_d.ap())
        nc.sync.dma_start(out=w1_t[:], in_=w1_d.ap())
        nc.sync.dma_start(out=wpos_t[:], in_=wpos_d.ap())
        nc.sync.dma_start(out=wps_t[:], in_=wps_d.ap())
        nc.sync.dma_start(out=wc_t[:], in_=wc_d.ap())
        nc.sync.dma_start(out=bias_t[:], in_=bias_d.ap())
        from concourse.masks import make_identity
        make_identity(nc, ident[:])
        nc.vector.memset(zeros_t[:], 0.0)

        def bias_ap(col):
            return bias_t[:, col:col + 1]

        # zero-init the output table: view [16384+1, 128] rows as
        # [128 part, 16 a, 8 chunks...] -> 8 chunked DMAs + last row
        tbl_main = table_d.ap()[0:R2, :].rearrange("(a p) f -> p a f", p=128)
        for i in range(32):
            nc.sync.dma_start(out=tbl_main[:, i * 4:(i + 1) * 4, :],
                              in_=zeros_t[:].rearrange("p (a f) -> p a f",
                                                       f=128))
        nc.sync.dma_start(out=table_d.ap()[R2:R2 + 1, :],
                          in_=zeros_t[0:1, 0:128])

        for seg in range(nseg):
            o = seg * SEG_M
            mm_s = segio.tile([128, SEG_M + 1], BF16, tag="maxmask")
            sm_s = segio.tile([128, SEG_M], BF16, tag="summask")
            wcol_s = segio.tile([128, NCH], FP32, tag="wcol")
            tg_s = segio.tile([128, NCH], I32, tag="targets")
            nc.sync.dma_start(out=mm_s[:], in_=mm_d.ap()[:, o:o + SEG_M + 1])
            nc.sync.dma_start(out=sm_s[:], in_=sm_d.ap()[:, o:o + SEG_M])
            nc.sync.dma_start(out=wcol_s[:],
                              in_=wcol_d.ap()[:, seg * NCH:(seg + 1) * NCH])
            nc.sync.dma_start(out=tg_s[:],
                              in_=tg_d.ap()[:, seg * NCH:(seg + 1) * NCH])

            netA = nets.tile([128, SEG_M], BF16, tag="netA")
            netB = nets.tile([128, SEG_M], BF16, tag="netB")

            # ---- block 0 (no pooling; shortcut folded through pos) ----
            for s in range(NSTRIP_SEG):
                sl = slice(s * STRIP, (s + 1) * STRIP)
                pos_s = work.tile([4, STRIP], BF16, tag="pos")
                nc.sync.dma_start(out=pos_s[:],
                                  in_=pos_d.ap()[:, o + s * STRIP:
                                                 o + (s + 1) * STRIP])
                ps_lo = psum.tile([128, STRIP], FP32, tag="ps_a")
                ps_hi = psum.tile([128, STRIP], FP32, tag="ps_b")
                nc.tensor.matmul(out=ps_lo[:], lhsT=wpos_t[:, 0:128],
                                 rhs=pos_s[:], start=True, stop=True)
                nc.tensor.matmul(out=ps_hi[:], lhsT=wpos_t[:, 128:256],
                                 rhs=pos_s[:], start=True, stop=True)
                rx_lo = work.tile([128, STRIP], BF16, tag="rx_lo")
                rx_hi = work.tile([128, STRIP], BF16, tag="rx_hi")
                nc.scalar.activation(out=rx_lo[:], in_=ps_lo[:], func=AF.Relu,
                                     bias=bias_ap(0))
                nc.scalar.activation(out=rx_hi[:], in_=ps_hi[:], func=AF.Relu,
                                     bias=bias_ap(1))
                ps_n = psum.tile([128, STRIP], FP32, tag="ps_a")
                nc.tensor.matmul(out=ps_n[:], lhsT=w0_t[:, 0:128],
                                 rhs=rx_lo[:], start=True, stop=False)
                nc.tensor.matmul(out=ps_n[:], lhsT=w0_t[:, 128:256],
                                 rhs=rx_hi[:], start=False, stop=True)
                rn = work.tile([128, STRIP], BF16, tag="rn")
                nc.scalar.activation(out=rn[:], in_=ps_n[:], func=AF.Relu,
                                     bias=bias_ap(2))
                ps_o = psum.tile([128, STRIP], FP32, tag="ps_b")
                nc.tensor.matmul(out=ps_o[:], lhsT=w1_t[:, 0:128],
                                 rhs=rn[:], start=True, stop=False)
                nc.tensor.matmul(out=ps_o[:], lhsT=wps_t[:],
                                 rhs=pos_s[:], start=False, stop=True)
                nc.scalar.activation(out=netA[:, sl], in_=ps_o[:],
                                     func=AF.Identity, bias=bias_ap(2 + NB))

            # ---- blocks 1..4 ----
            cur, nxt = netA, netB
            for blk in range(1, NB):
                fwd = scanp.tile([128, SEG_M], BF16, tag="fwd")
                pooled_seg = scanp.tile([128, SEG_M], BF16, tag="pooled_seg")
                nc.vector.tensor_tensor_scan(
                    out=fwd[:], data0=mm_s[:, 0:SEG_M], data1=cur[:],
                    initial=-BIG, op0=ALU.min, op1=ALU.max)
                # reverse running-max of the prefix-max == per-bin max at
                # every position: produces pooled directly
                nc.vector.tensor_tensor_scan(
                    out=pooled_seg[:, ::-1],
                    data0=mm_s[:, 1:SEG_M + 1][:, ::-1],
                    data1=fwd[:, ::-1],
                    initial=-BIG, op0=ALU.min, op1=ALU.max)
                c0 = 2 * blk * 128
                c1 = (2 * blk + 1) * 128
                for s in range(NSTRIP_SEG):
                    sl = slice(s * STRIP, (s + 1) * STRIP)
                    rp = work.tile([128, STRIP], BF16, tag="rp")
                    rn_prev = work.tile([128, STRIP], BF16, tag="rn_prev")
                    nc.scalar.activation(out=rp[:], in_=pooled_seg[:, sl],
                                         func=AF.Relu)
                    nc.scalar.activation(out=rn_prev[:], in_=cur[:, sl],
                                         func=AF.Relu)
                    ps_n = psum.tile([128, STRIP], FP32, tag="ps_a")
                    nc.tensor.matmul(out=ps_n[:], lhsT=w0_t[:, c0:c0 + 128],
                                     rhs=rn_prev[:], start=True, stop=False)
                    nc.tensor.matmul(out=ps_n[:], lhsT=w0_t[:, c1:c1 + 128],
                                     rhs=rp[:], start=False, stop=True)
                    rn = work.tile([128, STRIP], BF16, tag="rn")
                    nc.scalar.activation(out=rn[:], in_=ps_n[:], func=AF.Relu,
                                         bias=bias_ap(2 + blk))
                    ps_o = psum.tile([128, STRIP], FP32, tag="ps_b")
                    nc.tensor.matmul(out=ps_o[:],
                                     lhsT=w1_t[:, blk * 128:(blk + 1) * 128],
                                     rhs=rn[:], start=True, stop=False)
                    nc.tensor.matmul(out=ps_o[:], lhsT=ws_t[:, c0:c0 + 128],
                                     rhs=cur[:, sl], start=False, stop=False)
                    nc.tensor.matmul(out=ps_o[:], lhsT=ws_t[:, c1:c1 + 128],
                                     rhs=pooled_seg[:, sl], start=False,
                                     stop=True)
                    nc.scalar.activation(out=nxt[:, sl], in_=ps_o[:],
                                         func=AF.Identity,
                                         bias=bias_ap(2 + NB + blk))
                cur, nxt = nxt, cur

            # ---- final projection + segmented mean + scatter ----
            prev_csum = None
            for s in range(NSTRIP_SEG):
                sl = slice(s * STRIP, (s + 1) * STRIP)
                rn = work.tile([128, STRIP], BF16, tag="rn_f")
                nc.scalar.activation(out=rn[:], in_=cur[:, sl], func=AF.Relu)
                ps_c = psum.tile([128, STRIP], FP32, tag="ps_a")
                nc.tensor.matmul(out=ps_c[:], lhsT=wc_t[:], rhs=rn[:],
                                 start=True, stop=True)
                c_t = work.tile([128, STRIP], BF16, tag="c_t")
                nc.scalar.activation(out=c_t[:], in_=ps_c[:],
                                     func=AF.Identity,
                                     bias=bias_ap(2 + 2 * NB))
                csum = csump.tile([128, STRIP], FP32, tag="csum")
                nc.vector.tensor_tensor_scan(
                    out=csum[:], data0=sm_s[:, sl], data1=c_t[:],
                    initial=(0.0 if prev_csum is None
                             else prev_csum[:, STRIP - 1:STRIP]),
                    op0=ALU.mult, op1=ALU.add)
                prev_csum = csum
                for ch in range(4):
                    chg = s * 4 + ch
                    pt = psum_tr.tile([128, 128], FP32, tag="ptr")
                    nc.tensor.transpose(
                        out=pt[:], in_=csum[:, ch * 128:(ch + 1) * 128],
                        identity=ident[:])
                    srow = work.tile([128, 128], FP32, tag="srow")
                    nc.scalar.activation(out=srow[:], in_=pt[:],
                                         func=AF.Identity,
                                         scale=wcol_s[:, chg:chg + 1])
                    nc.gpsimd.indirect_dma_start(
                        out=table_d.ap(),
                        out_offset=bass.IndirectOffsetOnAxis(
                            ap=tg_s[:, chg:chg + 1], axis=0),
                        in_=srow[:], in_offset=None)

    nc.compile()
    return nc


# --------------------------------------------------------------------------
# public entry point
# --------------------------------------------------------------------------

_CACHED = {}


def kernel(inputs, Wpos, bpos, W0, b0, W1, b1, Ws, Wc, bc):
    W = dict(Wpos=np.asarray(Wpos, np.float32), bpos=np.asarray(bpos, np.float32),
             W0=np.asarray(W0, np.float32), b0=np.asarray(b0, np.float32),
             W1=np.asarray(W1, np.float32), b1=np.asarray(b1, np.float32),
             Ws=np.asarray(Ws, np.float32), Wc=np.asarray(Wc, np.float32),
             bc=np.asarray(bc, np.float32))
    inputs = np.asarray(inputs, np.float32)
    b = inputs.shape[0]

    cores = _host_prep(inputs)
    packed = _pack_weights(W)
    in_maps = [_make_core_inputs(c, packed) for c in cores]

    # bass_utils' axon trace path imports antenv.axon_hooks, which may be
    # missing on this image; provide a minimal stand-in so trace requests
    # degrade gracefully instead of raising.
    try:
        from antenv import axon_hooks  # noqa: F401
    except ImportError:
        import sys
        import types
        import antenv
        mod = types.ModuleType("antenv.axon_hooks")
        mod._hook = None
        mod.set_axon_ntff_profile_hook = lambda h: setattr(mod, "_hook", h)
        mod.get_axon_ntff_profile_hook = lambda: mod._hook
        sys.modules["antenv.axon_hooks"] = mod
        antenv.axon_hooks = mod

    from concourse.bass_utils import run_bass_kernel_spmd
    if "nc" not in _CACHED:
        _CACHED["nc"] = _build_kernel()
    nc = _CACHED["nc"]

    trace = bool(int(os.environ.get("KERNEL_TRACE", "0")))
    res = run_bass_kernel_spmd(nc, in_maps, core_ids=list(range(NCORES)),
                               trace=trace)
    if trace and res.exec_time_ns is not None:
        print(f"HW exec time: {res.exec_time_ns} ns")
        _CACHED["exec_time_ns"] = res.exec_time_ns
        _CACHED["trace"] = res.instructions_and_trace

    out = np.zeros((b, H, R2), np.float32)
    for core, r in zip(cores, res.results):
        table = np.asarray(r["table"])
        owned = core["owned"]
        out[core["sample"]][:, owned] = table[owned].T
    return out.reshape(b, H, R, R)


# revision 10
# speedup vs baseline: 3.0809x; 1.2713x over previous
"""Trainium2 Bass kernel for nn_LocalPoolPointnet (LocalPoolPointnet, 8 cores).

Strategy (chosen sharding): each of the 4 samples is split across 2 cores at a
scatter-bin boundary, after sorting that sample's points by flat bin index
(host-side prep, part of sharding). Each core therefore owns a set of COMPLETE
bins, so no cross-core communication is needed: local-max pooling and the final
scatter-mean are computed with segmented scans along the sorted point stream.

Device pipeline (feature-major: 128 features on partitions, points on free):
  - MLP resblocks as PE matmuls (bf16, fp32 PSUM accumulate)
  - pool_local_max = fwd+bwd masked max scans (tensor_tensor_scan,
    op0=min with +/-BIG segment mask, op1=max), pooled = max(fwd, bwd)
  - final scatter_mean = masked sum scan; at each bin's last point the running
    sum is the bin total; multiply by host-provided 1/count, transpose the
    chunk to point-major, and indirect-DMA the segment-end rows to the
    per-bin output table. Non-end rows go to a dummy row.
"""

import os
import numpy as np
from contextlib import ExitStack

R = 128
R2 = R * R
H = 128
NB = 5
BIG = 1e30

STRIP = 512
NSTRIP_SEG = int(os.environ.get("KERNEL_NSTRIP", 26))  # strips per segment
SEG_M = STRIP * NSTRIP_SEG    # 13312 points per segment
NSEG = int(os.environ.get("KERNEL_NSEG", 4))           # segments per core
N_CAP = SEG_M * NSEG          # per-core padded point capacity
NCH = SEG_M // 128            # 104 point-chunks per segment
NCORES = 8


# --------------------------------------------------------------------------
# host-side prep: sort by bin, split across cores/segments, build metadata
# --------------------------------------------------------------------------

def _host_prep(inputs):
    b, n, _ = inputs.shape
    cores = []
    for s in range(b):
        pts = np.asarray(inputs[s], np.float32)
        ij = np.clip((pts[:, :2] * R).astype(np.int32), 0, R - 1)
        idx = ij[:, 0] + R * ij[:, 1]
        order = np.argsort(idx, kind="stable")
        idx_s = idx[order]
        pts_s = pts[order]
        half = n // 2
        split_pos = int(np.searchsorted(idx_s, idx_s[half]))
        for (lo, hi) in [(0, split_pos), (split_pos, n)]:
            ci, cp = idx_s[lo:hi], pts_s[lo:hi]
            npts = hi - lo
            assert npts <= N_CAP, f"core overflow {npts} > {N_CAP}"
            seg_bounds = [0]
            for k in range(1, NSEG):
                tgt = min(npts - 1, round(npts * k / NSEG))
                seg_bounds.append(int(np.searchsorted(ci, ci[tgt])))
            seg_bounds.append(npts)

            pos = np.zeros((4, N_CAP), np.float32)
            maxmask = np.full((N_CAP + 1,), -BIG, np.float32)
            summask = np.zeros((N_CAP,), np.float32)
            wrow = np.zeros((N_CAP,), np.float32)
            targets = np.full((N_CAP,), R2, np.int32)
            for k in range(NSEG):
                a, e = seg_bounds[k], seg_bounds[k + 1]
                m = e - a
                assert m <= SEG_M, f"segment overflow {m} > {SEG_M}"
                o = k * SEG_M
                seg_idx = ci[a:e]
                pos[:3, o:o + m] = cp[a:e].T
                same = np.zeros(m, bool)
                same[1:] = seg_idx[1:] == seg_idx[:-1]
                maxmask[o:o + m] = np.where(same, BIG, -BIG)
                summask[o:o + m] = same.astype(np.float32)
                is_end = np.ones(m, bool)
                is_end[:-1] = seg_idx[1:] != seg_idx[:-1]
                _, inv, cnt = np.unique(seg_idx, return_inverse=True,
                                        return_counts=True)
                wrow[o:o + m] = np.where(is_end, 1.0 / cnt[inv], 0.0)
                targets[o:o + m] = np.where(is_end, seg_idx, R2)
            owned = np.unique(ci)
            cores.append(dict(sample=s, pos=pos, maxmask=maxmask,
                              summask=summask, wrow=wrow, targets=targets,
                              owned=owned))
    return cores


def _pack_weights(W):
    """Pack reference weights into the device layouts (bf16/fp32)."""
    bf = np.dtype("bfloat16") if hasattr(np, "bfloat16") else None
    import ml_dtypes
    bf16 = ml_dtypes.bfloat16

    w0 = np.zeros((128, NB * 2 * 128), np.float32)
    ws = np.zeros((128, NB * 2 * 128), np.float32)
    w1 = np.zeros((128, NB * 128), np.float32)
    for blk in range(NB):
        w0[:, (2 * blk) * 128:(2 * blk + 1) * 128] = W["W0"][blk][:128, :]
        w0[:, (2 * blk + 1) * 128:(2 * blk + 2) * 128] = W["W0"][blk][128:, :]
        ws[:, (2 * blk) * 128:(2 * blk + 1) * 128] = W["Ws"][blk][:128, :]
        ws[:, (2 * blk + 1) * 128:(2 * blk + 2) * 128] = W["Ws"][blk][128:, :]
        w1[:, blk * 128:(blk + 1) * 128] = W["W1"][blk]

    wpos = np.zeros((4, 256), np.float32)
    wpos[:3] = W["Wpos"]
    # block0 shortcut folded through the (linear) position encoder:
    # x0 @ Ws0 = pos @ (Wpos @ Ws0) + bpos @ Ws0
    wps = np.zeros((4, 128), np.float32)
    wps[:3] = W["Wpos"] @ W["Ws"][0]
    bps = W["bpos"] @ W["Ws"][0]

    biases = np.zeros((128, 2 + 2 * NB + 1), np.float32)
    biases[:, 0] = W["bpos"][:128]
    biases[:, 1] = W["bpos"][128:]
    for blk in range(NB):
        biases[:, 2 + blk] = W["b0"][blk]
        biases[:, 2 + NB + blk] = W["b1"][blk]
    biases[:, 2 + NB] += bps          # fold folded-shortcut bias into b1[0]
    biases[:, 2 + 2 * NB] = W["bc"]

    return dict(
        w0=w0.astype(bf16), ws=ws.astype(bf16), w1=w1.astype(bf16),
        wpos=wpos.astype(bf16), wps=wps.astype(bf16),
        wc=np.ascontiguousarray(W["Wc"]).astype(bf16),
        biases=biases,
    )


def _make_core_inputs(core, packed):
    import ml_dtypes
    bf16 = ml_dtypes.bfloat16
    mm = np.broadcast_to(core["maxmask"][None, :], (128, N_CAP + 1))
    sm = np.broadcast_to(core["summask"][None, :], (128, N_CAP))
    return dict(
        pos=core["pos"].astype(bf16),
        maxmask=np.ascontiguousarray(mm).astype(bf16),
        summask=np.ascontiguousarray(sm).astype(bf16),
        wcol=np.ascontiguousarray(
            core["wrow"].reshape(-1, 128).T).astype(np.float32),
        targets=np.ascontiguousarray(
            core["targets"].reshape(-1, 128).T).astype(np.int32),
        **packed,
    )


# --------------------------------------------------------------------------
# device kernel
# --------------------------------------------------------------------------

def _build_kernel(nseg=NSEG):
    import concourse.bass as bass
    import concourse.tile as tile
    import concourse.mybir as mybir
    from concourse import bacc

    BF16 = mybir.dt.bfloat16
    FP32 = mybir.dt.float32
    I32 = mybir.dt.int32
    AF = mybir.ActivationFunctionType
    ALU = mybir.AluOpType
    n_cap = SEG_M * nseg
    nch_total = (n_cap) // 128

    nc = bacc.Bacc("TRN2", target_bir_lowering=False, debug=False,
                   num_devices=NCORES)

    pos_d = nc.dram_tensor("pos", [4, n_cap], BF16, kind="ExternalInput")
    mm_d = nc.dram_tensor("maxmask", [128, n_cap + 1], BF16,
                          kind="ExternalInput")
    sm_d = nc.dram_tensor("summask", [128, n_cap], BF16, kind="ExternalInput")
    wcol_d = nc.dram_tensor("wcol", [128, nch_total], FP32,
                            kind="ExternalInput")
    tg_d = nc.dram_tensor("targets", [128, nch_total], I32,
                          kind="ExternalInput")
    w0_d = nc.dram_tensor("w0", [128, NB * 2 * 128], BF16,
                          kind="ExternalInput")
    ws_d = nc.dram_tensor("ws", [128, NB * 2 * 128], BF16,
                          kind="ExternalInput")
    w1_d = nc.dram_tensor("w1", [128, NB * 128], BF16, kind="ExternalInput")
    wpos_d = nc.dram_tensor("wpos", [4, 256], BF16, kind="ExternalInput")
    wps_d = nc.dram_tensor("wps", [4, 128], BF16, kind="ExternalInput")
    wc_d = nc.dram_tensor("wc", [128, 128], BF16, kind="ExternalInput")
    bias_d = nc.dram_tensor("biases", [128, 2 + 2 * NB + 1], FP32,
                            kind="ExternalInput")
    table_d = nc.dram_tensor("table", [R2 + 1, 128], FP32,
                             kind="ExternalOutput")

    with tile.TileContext(nc) as tc, ExitStack() as ctx:
        const = ctx.enter_context(tc.tile_pool(name="const", bufs=1))
        segio = ctx.enter_context(tc.tile_pool(name="segio", bufs=1))
        nets = ctx.enter_context(tc.tile_pool(name="nets", bufs=1))
        scanp = ctx.enter_context(tc.tile_pool(name="scanp", bufs=1))
        work = ctx.enter_context(tc.tile_pool(name="work", bufs=2))
        csump = ctx.enter_context(tc.tile_pool(name="csum", bufs=3))
        psum = ctx.enter_context(tc.tile_pool(name="psum", bufs=3,
                                              space="PSUM"))
        psum_tr = ctx.enter_context(tc.tile_pool(name="psum_tr", bufs=2,
                                                 space="PSUM"))

        # ---- constants ----
        w0_t = const.tile([128, NB * 2 * 128], BF16)
        ws_t = const.tile([128, NB * 2 * 128], BF16)
        w1_t = const.tile([128, NB * 128], BF16)
        wpos_t = const.tile([4, 256], BF16)
        wps_t = const.tile([4, 128], BF16)
        wc_t = const.tile([128, 128], BF16)
        bias_t = const.tile([128, 2 + 2 * NB + 1], FP32)
        ident = const.tile([128, 128], FP32)
        zeros_t = const.tile([128, 512], FP32)
        nc.sync.dma_start(out=w0_t[:], in_=w0_d.ap())
        nc.sync.dma_start(out=ws_t[:], in_=ws_d.ap())
        nc.sync.dma_start(out=w1_t[:], in_=w1_d.ap())
        nc.sync.dma_start(out=wpos_t[:], in_=wpos_d.ap())
        nc.sync.dma_start(out=wps_t[:], in_=wps_d.ap())
        nc.sync.dma_start(out=wc_t[:], in_=wc_d.ap())
        nc.sync.dma_start(out=bias_t[:], in_=bias_d.ap())
        from concourse.masks import make_identity
        make_identity(nc, ident[:])
        nc.vector.memset(zeros_t[:], 0.0)

        def bias_ap(col):
            return bias_t[:, col:col + 1]

        # zero-init the output table: view [16384+1, 128] rows as
        # [128 part, 16 a, 8 chunks...] -> 8 chunked DMAs + last row
        tbl_main = table_d.ap()[0:R2, :].rearrange("(a p) f -> p a f", p=128)
        for i in range(32):
            nc.sync.dma_start(out=tbl_main[:, i * 4:(i + 1) * 4, :],
                              in_=zeros_t[:].rearrange("p (a f) -> p a f",
                                                       f=128))
        nc.sync.dma_start(out=table_d.ap()[R2:R2 + 1, :],
                          in_=zeros_t[0:1, 0:128])

        ND = NSTRIP_SEG

        def startflags(mm_s, st):
            return mm_s[:, st * STRIP:(st + 1) * STRIP]

        def endflags(mm_s, st):
            return mm_s[:, st * STRIP + 1:(st + 1) * STRIP + 1]

        for seg in range(nseg):
            o = seg * SEG_M
            mm_s = segio.tile([128, SEG_M + 1], BF16, tag="maxmask")
            wcol_s = segio.tile([128, NCH], FP32, tag="wcol")
            tg_s = segio.tile([128, NCH], I32, tag="targets")
            nc.sync.dma_start(out=mm_s[:], in_=mm_d.ap()[:, o:o + SEG_M + 1])
            nc.sync.dma_start(out=wcol_s[:],
                              in_=wcol_d.ap()[:, seg * NCH:(seg + 1) * NCH])
            nc.sync.dma_start(out=tg_s[:],
                              in_=tg_d.ap()[:, seg * NCH:(seg + 1) * NCH])

            # per-strip paired tiles: [:, 0:STRIP] = net, [:, STRIP:] = pooled
            cpA = [nets.tile([128, 2 * STRIP], BF16, tag=f"cpA{st}",
                             name=f"cpA{st}") for st in range(ND)]
            cpB = [nets.tile([128, 2 * STRIP], BF16, tag=f"cpB{st}",
                             name=f"cpB{st}") for st in range(ND)]
            sA = [nets.tile([128, STRIP], BF16, tag=f"sA{st}",
                            name=f"sA{st}") for st in range(ND)]

            def emit_scanA(cur, st, d, last):
                """running-max chunk over net (direction d), out sA[st].
                last = True when this is the first chunk of the chain."""
                if d == 0:
                    nc.vector.tensor_tensor_scan(
                        out=sA[st][:], data0=startflags(mm_s, st),
                        data1=cur[st][:, 0:STRIP],
                        initial=(-BIG if st == 0
                                 else sA[st - 1][:, STRIP - 1:STRIP]),
                        op0=ALU.min, op1=ALU.max)
                else:
                    nc.vector.tensor_tensor_scan(
                        out=sA[st][:, ::-1],
                        data0=endflags(mm_s, st)[:, ::-1],
                        data1=cur[st][:, 0:STRIP][:, ::-1],
                        initial=(-BIG if st == ND - 1
                                 else sA[st + 1][:, 0:1]),
                        op0=ALU.min, op1=ALU.max)

            def emit_scanB(cur, st, d):
                """second-direction chunk over sA -> per-bin max, written to
                cur[st]'s pooled half. d = direction of THIS scan."""
                dst = cur[st][:, STRIP:2 * STRIP]
                if d == 0:
                    nc.vector.tensor_tensor_scan(
                        out=dst, data0=startflags(mm_s, st), data1=sA[st][:],
                        initial=(-BIG if st == 0
                                 else cur[st - 1][:, 2 * STRIP - 1:2 * STRIP]),
                        op0=ALU.min, op1=ALU.max)
                else:
                    nc.vector.tensor_tensor_scan(
                        out=dst[:, ::-1],
                        data0=endflags(mm_s, st)[:, ::-1],
                        data1=sA[st][:, ::-1],
                        initial=(-BIG if st == ND - 1
                                 else cur[st + 1][:, STRIP:STRIP + 1]),
                        op0=ALU.min, op1=ALU.max)

            # ---- block 0 (forward; shortcut folded through pos) ----
            for st in range(ND):
                pos_s = work.tile([4, STRIP], BF16, tag="pos")
                nc.sync.dma_start(out=pos_s[:],
                                  in_=pos_d.ap()[:, o + st * STRIP:
                                                 o + (st + 1) * STRIP])
                ps_lo = psum.tile([128, STRIP], FP32, tag="ps_a")
                ps_hi = psum.tile([128, STRIP], FP32, tag="ps_b")
                nc.tensor.matmul(out=ps_lo[:], lhsT=wpos_t[:, 0:128],
                                 rhs=pos_s[:], start=True, stop=True)
                nc.tensor.matmul(out=ps_hi[:], lhsT=wpos_t[:, 128:256],
                                 rhs=pos_s[:], start=True, stop=True)
                rx_lo = work.tile([128, STRIP], BF16, tag="rx_lo")
                rx_hi = work.tile([128, STRIP], BF16, tag="rx_hi")
                nc.scalar.activation(out=rx_lo[:], in_=ps_lo[:], func=AF.Relu,
                                     bias=bias_ap(0))
                nc.scalar.activation(out=rx_hi[:], in_=ps_hi[:], func=AF.Relu,
                                     bias=bias_ap(1))
                ps_n = psum.tile([128, STRIP], FP32, tag="ps_a")
                nc.tensor.matmul(out=ps_n[:], lhsT=w0_t[:, 0:128],
                                 rhs=rx_lo[:], start=True, stop=False)
                nc.tensor.matmul(out=ps_n[:], lhsT=w0_t[:, 128:256],
                                 rhs=rx_hi[:], start=False, stop=True)
                rn = work.tile([128, STRIP], BF16, tag="rn")
                nc.scalar.activation(out=rn[:], in_=ps_n[:], func=AF.Relu,
                                     bias=bias_ap(2))
                ps_o = psum.tile([128, STRIP], FP32, tag="ps_b")
                nc.tensor.matmul(out=ps_o[:], lhsT=w1_t[:, 0:128],
                                 rhs=rn[:], start=True, stop=False)
                nc.tensor.matmul(out=ps_o[:], lhsT=wps_t[:],
                                 rhs=pos_s[:], start=False, stop=True)
                nc.scalar.activation(out=cpA[st][:, 0:STRIP], in_=ps_o[:],
                                     func=AF.Identity, bias=bias_ap(2 + NB))
                emit_scanA(cpA, st, 0, st == ND - 1)

            # ---- blocks 1..4 (alternating strip direction) ----
            cur, nxt = cpA, cpB
            for blk in range(1, NB):
                d = blk % 2           # 1 = reverse strips, 0 = forward
                c0 = 2 * blk * 128
                c1 = (2 * blk + 1) * 128
                order = range(ND - 1, -1, -1) if d else range(ND)
                for st in order:
                    emit_scanB(cur, st, d)
                    rx = work.tile([128, 2 * STRIP], BF16, tag="rx")
                    nc.scalar.activation(out=rx[:], in_=cur[st][:],
                                         func=AF.Relu)
                    ps_n = psum.tile([128, STRIP], FP32, tag="ps_a")
                    nc.tensor.matmul(out=ps_n[:], lhsT=w0_t[:, c0:c0 + 128],
                                     rhs=rx[:, 0:STRIP], start=True,
                                     stop=False)
                    nc.tensor.matmul(out=ps_n[:], lhsT=w0_t[:, c1:c1 + 128],
                                     rhs=rx[:, STRIP:2 * STRIP], start=False,
                                     stop=True)
                    rn = work.tile([128, STRIP], BF16, tag="rn")
                    nc.vector.tensor_scalar(
                        out=rn[:], in0=ps_n[:], scalar1=bias_ap(2 + blk),
                        scalar2=0.0, op0=ALU.add, op1=ALU.max)
                    ps_o = psum.tile([128, STRIP], FP32, tag="ps_b")
                    nc.tensor.matmul(out=ps_o[:],
                                     lhsT=w1_t[:, blk * 128:(blk + 1) * 128],
                                     rhs=rn[:], start=True, stop=False)
                    nc.tensor.matmul(out=ps_o[:], lhsT=ws_t[:, c0:c0 + 128],
                                     rhs=cur[st][:, 0:STRIP], start=False,
                                     stop=False)
                    nc.tensor.matmul(out=ps_o[:], lhsT=ws_t[:, c1:c1 + 128],
                                     rhs=cur[st][:, STRIP:2 * STRIP],
                                     start=False, stop=True)
                    nc.scalar.activation(out=nxt[st][:, 0:STRIP], in_=ps_o[:],
                                         func=AF.Identity,
                                         bias=bias_ap(2 + NB + blk))
                    if blk < NB - 1:
                        emit_scanA(nxt, st, d, False)
                cur, nxt = nxt, cur

            # ---- final projection + segmented mean + scatter (forward) ----
            prev_csum = None
            for st in range(ND):
                sl = slice(st * STRIP, (st + 1) * STRIP)
                rn = work.tile([128, STRIP], BF16, tag="rn_f")
                nc.scalar.activation(out=rn[:], in_=cur[st][:, 0:STRIP],
                                     func=AF.Relu)
                ps_c = psum.tile([128, STRIP], FP32, tag="ps_a")
                nc.tensor.matmul(out=ps_c[:], lhsT=wc_t[:], rhs=rn[:],
                                 start=True, stop=True)
                c_t = work.tile([128, STRIP], BF16, tag="c_t")
                nc.scalar.activation(out=c_t[:], in_=ps_c[:],
                                     func=AF.Identity,
                                     bias=bias_ap(2 + 2 * NB))
                sm_t = work.tile([128, STRIP], BF16, tag="sm_t")
                nc.sync.dma_start(out=sm_t[:],
                                  in_=sm_d.ap()[:, o + st * STRIP:
                                                o + (st + 1) * STRIP])
                csum = csump.tile([128, STRIP], FP32, tag="csum")
                nc.vector.tensor_tensor_scan(
                    out=csum[:], data0=sm_t[:], data1=c_t[:],
                    initial=(0.0 if prev_csum is None
                             else prev_csum[:, STRIP - 1:STRIP]),
                    op0=ALU.mult, op1=ALU.add)
                prev_csum = csum
                for ch in range(4):
                    chg = st * 4 + ch
                    pt = psum_tr.tile([128, 128], FP32, tag="ptr")
                    nc.tensor.transpose(
                        out=pt[:], in_=csum[:, ch * 128:(ch + 1) * 128],
                        identity=ident[:])
                    srow = work.tile([128, 128], FP32, tag="srow")
                    nc.scalar.activation(out=srow[:], in_=pt[:],
                                         func=AF.Identity,
                                         scale=wcol_s[:, chg:chg + 1])
                    nc.gpsimd.indirect_dma_start(
                        out=table_d.ap(),
                        out_offset=bass.IndirectOffsetOnAxis(
                            ap=tg_s[:, chg:chg + 1], axis=0),
                        in_=srow[:], in_offset=None)

    nc.compile()
    return nc


# --------------------------------------------------------------------------
# public entry point
# --------------------------------------------------------------------------

_CACHED = {}


def kernel(inputs, Wpos, bpos, W0, b0, W1, b1, Ws, Wc, bc):
    W = dict(Wpos=np.asarray(Wpos, np.float32), bpos=np.asarray(bpos, np.float32),
             W0=np.asarray(W0, np.float32), b0=np.asarray(b0, np.float32),
             W1=np.asarray(W1, np.float32), b1=np.asarray(b1, np.float32),
             Ws=np.asarray(Ws, np.float32), Wc=np.asarray(Wc, np.float32),
             bc=np.asarray(bc, np.float32))
    inputs = np.asarray(inputs, np.float32)
    b = inputs.shape[0]

    cores = _host_prep(inputs)
    packed = _pack_weights(W)
    in_maps = [_make_core_inputs(c, packed) for c in cores]

    # bass_utils' axon trace path imports antenv.axon_hooks, which may be
    # missing on this image; provide a minimal stand-in so trace requests
    # degrade gracefully instead of raising.
    try:
        from antenv import axon_hooks  # noqa: F401
    except ImportError:
        import sys
        import types
        import antenv
        mod = types.ModuleType("antenv.axon_hooks")
        mod._hook = None
        mod.set_axon_ntff_profile_hook = lambda h: setattr(mod, "_hook", h)
        mod.get_axon_ntff_profile_hook = lambda: mod._hook
        sys.modules["antenv.axon_hooks"] = mod
        antenv.axon_hooks = mod

    from concourse.bass_utils import run_bass_kernel_spmd
    if "nc" not in _CACHED:
        _CACHED["nc"] = _build_kernel()
    nc = _CACHED["nc"]

    trace = bool(int(os.environ.get("KERNEL_TRACE", "0")))
    res = run_bass_kernel_spmd(nc, in_maps, core_ids=list(range(NCORES)),
                               trace=trace)
    if trace and res.exec_time_ns is not None:
        print(f"HW exec time: {res.exec_time_ns} ns")
        _CACHED["exec_time_ns"] = res.exec_time_ns
        _CACHED["trace"] = res.instructions_and_trace

    out = np.zeros((b, H, R2), np.float32)
    for core, r in zip(cores, res.results):
        table = np.asarray(r["table"])
        owned = core["owned"]
        out[core["sample"]][:, owned] = table[owned].T
    return out.reshape(b, H, R, R)
